# revision 24
# baseline (speedup 1.0000x reference)
"""Distributed Trainium2 kernel for nn_AssociativeMemoryBank.

Math (reference):
  q = query @ Wq^T + bq                       [B, H]    -> heads [B, 8, 64]
  k = MK @ Wk^T + bk ; v = MV @ Wv^T + bv     [C, H]
  scores = q.k / 8 ; attn = softmax_c(scores) [B, 8, C]
  ctx = attn @ v ; attn_out = ctx @ Wo^T + bo [B, H]
  avg_attn = attn.mean(b, h)                  [C]
  new_usage = usage + avg_attn; new_last = where(avg_attn > 1e-3, step, last)
  out = sigmoid(relu(query@Gw1^T+g1)@Gw2^T+g2) * attn_out

Sharding: capacity axis (C=65536) split across 8 cores (8192 each).
Per core, one pass over the K-shard computes E^T = exp(scores^T) ([c,(h,b)],
SBUF-resident bf16; scores are ~N(0, 0.002) so no max-subtraction is needed),
plus the local sumexp via a ones-vector matmul.  AllReduce #1 (4KB) combines
sumexp.  A second pass over the V-shard accumulates the unnormalized
attn-weighted value sums u = E^T-weighted V (PSUM-resident) and the
1/sumexp-weighted per-slot attention column sums (usage stats).  AllReduce #2
(256KB) combines u; the tiny output projection + gate MLP run replicated.

Note bk cancels exactly (it shifts each (b,h) score row by a constant, which
softmax removes), so it is dropped.  bv enters as +bv (attn sums to 1).

Host-side prep is layout/dtype only (shard, transpose, bf16-cast); all FLOPs
over the big tensors run on device.
"""

import numpy as np
import ml_dtypes

B = 128
CAP = 65536
H = 512
NH = 8
HD = 64
GH = 128
NCORES = 8
CS = CAP // NCORES       # 8192 capacity slots per core
NCHUNK = CS // 128       # 64  c-chunks of 128
NBIG = CS // 512         # 16  c-chunks of 512
BF16 = ml_dtypes.bfloat16
FP8 = ml_dtypes.float8_e4m3

_CACHE = {}


def _build(stage=9):
    import concourse.bacc as bacc
    import concourse.mybir as mybir
    import concourse.tile as tile

    dt = mybir.dt
    F32 = dt.float32
    BF = dt.bfloat16
    F8 = dt.float8e4
    AF = mybir.ActivationFunctionType
    RG = [list(range(NCORES))]

    nc = bacc.Bacc(
        "TRN2", target_bir_lowering=False, debug=False, num_devices=NCORES
    )

    # ---- external inputs (per-core shards / replicated smalls) ----
    mk8 = nc.declare_dram_parameter("mk8", [NBIG, 2, 128, 2, 512], F8, isOutput=False)
    mvT = nc.declare_dram_parameter("mvT", [NBIG, 4, 128, 512], BF, isOutput=False)
    queryT = nc.declare_dram_parameter("queryT", [4, 128, B], BF, isOutput=False)
    wqT = nc.declare_dram_parameter("wqT", [4, 128, H], BF, isOutput=False)
    wkN = nc.declare_dram_parameter("wkN", [4, 128, H], BF, isOutput=False)
    wvT = nc.declare_dram_parameter("wvT", [4, 128, H], BF, isOutput=False)
    woT = nc.declare_dram_parameter("woT", [4, 128, H], BF, isOutput=False)
    gw1T = nc.declare_dram_parameter("gw1T", [4, 128, GH], BF, isOutput=False)
    gw2T = nc.declare_dram_parameter("gw2T", [GH, 1], BF, isOutput=False)
    bq = nc.declare_dram_parameter("bq", [4, 128, 1], F32, isOutput=False)
    bv = nc.declare_dram_parameter("bv", [4, 128, 1], F32, isOutput=False)
    bo = nc.declare_dram_parameter("bo", [4, 128, 1], F32, isOutput=False)
    gb1 = nc.declare_dram_parameter("gb1", [GH, 1], F32, isOutput=False)
    gb2 = nc.declare_dram_parameter("gb2", [1, 1], F32, isOutput=False)
    stepc = nc.declare_dram_parameter("stepc", [1, 1], F32, isOutput=False)
    usage = nc.declare_dram_parameter("usage", [128, NCHUNK], F32, isOutput=False)
    last = nc.declare_dram_parameter("last", [128, NCHUNK], F32, isOutput=False)

    out_mo = nc.declare_dram_parameter("out_mo", [128, H], F32, isOutput=True)
    out_us = nc.declare_dram_parameter("out_us", [128, NCHUNK], F32, isOutput=True)
    out_la = nc.declare_dram_parameter("out_la", [128, NCHUNK], F32, isOutput=True)

    with tile.TileContext(nc) as tc:
        with (
            tc.tile_pool(name="const", bufs=1) as const,
            tc.tile_pool(name="dram", bufs=1, space="DRAM") as dram,
        ):
            # ---------- persistent SBUF ----------
            e_sb = const.tile([128, NCHUNK * 1024], BF, tag="E")   # E^T, 128KB/p
            wkN_sb = const.tile([128, 4 * H], BF, tag="wkN")
            a8_sb = const.tile([128, 4 * NH * B], F8, tag="a8")
            wv_sb = const.tile([128, 4 * H], BF, tag="wv")
            wo_sb = const.tile([128, 4 * H], BF, tag="wo")
            gw1_sb = const.tile([128, 4 * GH], BF, tag="gw1")
            gw2_sb = const.tile([GH, 1], BF, tag="gw2")
            qry_sb = const.tile([128, 4 * B], BF, tag="qry")
            qT_sb = const.tile([128, 4 * B], BF, tag="qT")
            qbd2_sb = const.tile([128, 4 * 256], BF, tag="qbd2")
            ones_col = const.tile([128, 1], BF, tag="ones")
            bq_sb = const.tile([128, 4], F32, tag="bq")
            bq8_sb = const.tile([128, 4], F32, tag="bq8")
            bv_sb = const.tile([128, 4], F32, tag="bv")
            bo_sb = const.tile([128, 4], F32, tag="bo")
            gb1_sb = const.tile([GH, 1], F32, tag="gb1")
            gb2_sb = const.tile([1, 1], F32, tag="gb2")
            step_sb = const.tile([1, 1], F32, tag="step")
            us_sb = const.tile([128, NCHUNK], F32, tag="us")
            la_sb = const.tile([128, NCHUNK], F32, tag="la")
            colsum_sb = const.tile([128, NCHUNK], F32, tag="colsum")
            esum_sb = const.tile([1, NH * B], F32, tag="esum")
            esumg_sb = const.tile([1, NH * B], F32, tag="esumg")
            recip_sb = const.tile([1, NH * B], F32, tag="recip")
            recip_rep = const.tile([128, NH * B], F32, tag="reciprep")
            w_rep = const.tile([128, NH * B], BF, tag="wrep")
            u_sb = const.tile([128, H], BF, tag="u")
            u2_sb = const.tile([128, H], BF, tag="u2")
            uga_sb = const.tile([128, H], BF, tag="uga")
            ugb_sb = const.tile([128, H], BF, tag="ugb")
            ug_sb = const.tile([128, H], BF, tag="ug")
            ctxf_sb = const.tile([128, H], F32, tag="ctxf")
            ctx_sb = const.tile([128, H], BF, tag="ctx")
            ao_sb = const.tile([128, H], F32, tag="ao")
            mo_sb = const.tile([128, H], F32, tag="mo")
            g1_sb = const.tile([GH, B], BF, tag="g1")
            gate_sb = const.tile([1, B], F32, tag="gate")
            gate_rep = const.tile([128, B], F32, tag="gaterep")
            mask_sb = const.tile([128, NCHUNK], dt.uint8, tag="mask")
            orow_sb = const.tile([1, NCHUNK], F32, tag="orow")
            srow_sb = const.tile([1, NCHUNK], F32, tag="srow")
            srep_sb = const.tile([128, NCHUNK], F32, tag="srep")
            ol_sb = const.tile([128, NCHUNK], F32, tag="ol")
            ou_sb = const.tile([128, NCHUNK], F32, tag="ou")
            cs_scratch = const.tile([128, NH * B], BF, tag="csscratch")
            cs_scratch2 = const.tile([128, NH * B], BF, tag="csscratch2")
            cs_scratch3 = const.tile([128, NH * B], BF, tag="csscratch3")

            # DRAM bounce buffers for the collectives
            esum_in = dram.tile([1, NH * B], F32)
            esum_out = dram.tile([1, NH * B], F32, addr_space="Shared")
            u_in = dram.tile([128, H], BF)
            u_out = dram.tile([128, H], BF, addr_space="Shared")
            u2_in = dram.tile([128, H], BF)
            u2_out = dram.tile([128, H], BF, addr_space="Shared")

            # ---------- load statics (SWDGE queue; HWDGE leads with K chunks) --
            for dc in range(4):
                nc.sync.dma_start(wkN_sb[:, dc * H:(dc + 1) * H], wkN[dc])
            for jc in range(4):
                nc.sync.dma_start(qry_sb[:, jc * B:(jc + 1) * B], queryT[jc])
                nc.sync.dma_start(bq_sb[:, jc:jc + 1], bq[jc])
                nc.gpsimd.dma_start(wv_sb[:, jc * H:(jc + 1) * H], wvT[jc])
                nc.gpsimd.dma_start(wo_sb[:, jc * H:(jc + 1) * H], woT[jc])
                nc.gpsimd.dma_start(gw1_sb[:, jc * GH:(jc + 1) * GH], gw1T[jc])
                nc.gpsimd.dma_start(bv_sb[:, jc:jc + 1], bv[jc])
                nc.gpsimd.dma_start(bo_sb[:, jc:jc + 1], bo[jc])
            nc.gpsimd.dma_start(gw2_sb[:], gw2T[:])
            nc.gpsimd.dma_start(gb1_sb[:], gb1[:])
            nc.gpsimd.dma_start(gb2_sb[:], gb2[:])
            nc.gpsimd.dma_start(step_sb[:], stepc[:])
            nc.gpsimd.dma_start(us_sb[:], usage[:])
            nc.gpsimd.dma_start(la_sb[:], last[:])
            nc.vector.memset(ones_col[:], 1.0)
            nc.vector.memset(orow_sb[:], 1.0)
            nc.vector.tensor_scalar_mul(bq8_sb[:], bq_sb[:], 0.125)

            # ---------- setup: q^T = (Wq @ query^T + bq)/8, bf16 ----------
            with (
                tc.tile_pool(name="psq", bufs=2, space="PSUM") as psq,
                tc.tile_pool(name="setup_sb", bufs=1) as setup_sb,
            ):
                wq_sb = setup_sb.tile([128, 4 * H], BF, tag="wq")
                for jc in range(4):
                    nc.sync.dma_start(
                        wq_sb[:, jc * H:(jc + 1) * H], wqT[jc]
                    )
                for dc in range(4):
                    q_ps = psq.tile([128, B], F32, tag="qps")
                    for jc in range(4):
                        nc.tensor.matmul(
                            q_ps[:],
                            lhsT=wq_sb[:, jc * H + dc * 128: jc * H + dc * 128 + 128],
                            rhs=qry_sb[:, jc * B:(jc + 1) * B],
                            start=(jc == 0),
                            stop=(jc == 3),
                        )
                    nc.scalar.activation(
                        qT_sb[:, dc * B:(dc + 1) * B],
                        q_ps[:],
                        AF.Identity,
                        bias=bq8_sb[:, dc:dc + 1],
                        scale=0.125,
                    )
            # block-diag packed q for head pairs: pair t cols [t*256, t*256+256)
            nc.vector.memset(qbd2_sb[:], 0.0)
            for p in range(4):
                nc.vector.tensor_copy(
                    qbd2_sb[0:64, p * 256: p * 256 + B],
                    qT_sb[0:64, p * B:(p + 1) * B],
                )
                nc.vector.tensor_copy(
                    qbd2_sb[64:128, p * 256 + B: p * 256 + 2 * B],
                    qT_sb[64:128, p * B:(p + 1) * B],
                )
            # A^T[j, (h,b)] = (Wk^T q)/8 : rows j, cols (h, b); fp8 x32
            with tc.tile_pool(name="psa", bufs=2, space="PSUM") as psa:
                for jc in range(4):
                    a_ps = psa.tile([128, NH * B], F32, tag="aps")
                    for t in range(4):
                        nc.tensor.matmul(
                            a_ps[:, t * 256:(t + 1) * 256],
                            lhsT=wkN_sb[:, t * H + jc * 128: t * H + jc * 128 + 128],
                            rhs=qbd2_sb[:, t * 256:(t + 1) * 256],
                            start=True,
                            stop=True,
                        )
                    nc.vector.tensor_scalar_mul(
                        a8_sb[:, jc * NH * B:(jc + 1) * NH * B], a_ps[:], 32.0
                    )

            # ---------- phase B1: K-projection, scores^T, E^T, local sumexp ----
            with (
                tc.tile_pool(name="pmk", bufs=3) as pmk,
                tc.tile_pool(name="pss", bufs=3, space="PSUM") as pss,
                tc.tile_pool(name="pse", bufs=1, space="PSUM") as pse,
            ):
                esum_ps = pse.tile([1, NH * B], F32, tag="esumps")
                for cc in range(NBIG):
                    mk_sb = pmk.tile([128, 2 * 1024], F8, tag="mk")
                    for jp in range(2):
                        nc.sync.dma_start(
                            mk_sb[:, jp * 1024:(jp + 1) * 1024], mk8[cc, jp]
                        )
                    for ci4 in range(4):
                        ci = cc * 4 + ci4
                        s_ps = pss.tile([128, 1024], F32, tag="sps")
                        for hf in range(2):
                            for jp in range(2):
                                nc.tensor.matmul(
                                    s_ps[:, hf * 512:(hf + 1) * 512],
                                    lhsT=mk_sb[
                                        :, jp * 1024:(jp + 1) * 1024
                                    ].rearrange("p (s c) -> p s c", s=2)[
                                        :, :, ci4 * 128:(ci4 + 1) * 128
                                    ],
                                    rhs=a8_sb.rearrange(
                                        "p (j s n) -> p j s n", j=2, s=2
                                    )[:, jp, :, hf * 512:(hf + 1) * 512],
                                    start=(jp == 0),
                                    stop=(jp == 1),
                                    perf_mode=mybir.MatmulPerfMode.DoubleRow,
                                )
                        nc.scalar.activation(
                            e_sb[:, ci * 1024:(ci + 1) * 1024],
                            s_ps[:],
                            AF.Exp,
                            scale=1.0 / 1024.0,
                        )
                        for hf in range(2):
                            nc.tensor.matmul(
                                esum_ps[:, hf * 512: hf * 512 + 512],
                                lhsT=ones_col[:],
                                rhs=e_sb[
                                    :,
                                    ci * 1024 + hf * 512: ci * 1024 + hf * 512 + 512,
                                ],
                                start=(ci == 0),
                                stop=(ci == NCHUNK - 1),
                                skip_group_check=True,
                            )
                nc.vector.tensor_copy(esum_sb[:], esum_ps[:])

            if stage < 2:
                nc.vector.memset(mo_sb[:], 0.0)
                nc.vector.tensor_copy(mo_sb[0:1, 0:H], esum_sb[:, 0:H])
                nc.sync.dma_start(out_mo[:], mo_sb[:])
                nc.vector.memset(ou_sb[:], 0.0)
                nc.sync.dma_start(out_us[:], ou_sb[:])
                nc.sync.dma_start(out_la[:], ou_sb[:])

            if stage >= 2:
                # ---------- AllReduce #1: sumexp ----------
                nc.sync.dma_start(esum_in[:], esum_sb[:])
                nc.gpsimd.collective_compute(
                    "AllReduce",
                    mybir.AluOpType.add,
                    replica_groups=RG,
                    ins=[esum_in.opt()],
                    outs=[esum_out.opt()],
                )
                nc.sync.dma_start(esumg_sb[:], esum_out[:])
                nc.vector.reciprocal(recip_sb[:], esumg_sb[:])
                nc.gpsimd.partition_broadcast(recip_rep[:], recip_sb[:])
                nc.vector.tensor_scalar_mul(w_rep[:], recip_rep[:], 1.0 / (B * NH))

            if stage == 2:
                nc.vector.memset(mo_sb[:], 0.0)
                nc.vector.tensor_copy(mo_sb[0:1, 0:H], esumg_sb[:, 0:H])
                nc.sync.dma_start(out_mo[:], mo_sb[:])
                nc.vector.memset(ou_sb[:], 0.0)
                nc.sync.dma_start(out_us[:], ou_sb[:])
                nc.sync.dma_start(out_la[:], ou_sb[:])

            if stage >= 3:
                # ---------- phase B2: V-projection, u accum, colsum ----------
                with (
                    tc.tile_pool(name="pmv", bufs=3) as pmv,
                    tc.tile_pool(name="pvt", bufs=3) as pvt,
                    tc.tile_pool(name="psv", bufs=2, space="PSUM") as psv,
                    tc.tile_pool(name="psu", bufs=1, space="PSUM") as psu,
                ):
                    u_ps = [
                        psu.tile([128, B], F32, tag=f"ups{t}", name=f"ups{t}")
                        for t in range(4)
                    ]
                    for cc in range(NBIG):
                        mv_sb = pmv.tile([128, 4 * 512], BF, tag="mv")
                        for jc in range(4):
                            nc.sync.dma_start(
                                mv_sb[:, jc * 512:(jc + 1) * 512], mvT[cc, jc]
                            )
                        for ci4 in range(4):
                            ci = cc * 4 + ci4
                            v_ps = psv.tile([128, 512], F32, tag="vps")
                            for jc in range(4):
                                nc.tensor.matmul(
                                    v_ps[:],
                                    lhsT=mv_sb[
                                        :,
                                        jc * 512 + ci4 * 128: jc * 512 + ci4 * 128
                                        + 128,
                                    ],
                                    rhs=wv_sb[:, jc * H:(jc + 1) * H],
                                    start=(jc == 0),
                                    stop=(jc == 3),
                                )
                            v_sb = pvt.tile([128, 512], BF, tag="vt")
                            nc.scalar.copy(v_sb[:], v_ps[:])
                            usplit = max(1, (NCHUNK * 3) // 4)
                            for h in range(NH):
                                nc.tensor.matmul(
                                    u_ps[h // 2][(h % 2) * 64:(h % 2) * 64 + 64, :],
                                    lhsT=v_sb[:, h * 64: h * 64 + 64],
                                    rhs=e_sb[
                                        :, ci * 1024 + h * B: ci * 1024 + (h + 1) * B
                                    ],
                                    start=(ci == 0 or ci == usplit),
                                    stop=(ci == usplit - 1 or ci == NCHUNK - 1),
                                    skip_group_check=True,
                                )
                            if ci == usplit - 1:
                                # first partial AllReduce, overlapped with the rest
                                for t in range(4):
                                    nc.scalar.copy(
                                        u_sb[:, t * B:(t + 1) * B], u_ps[t][:]
                                    )
                                nc.sync.dma_start(u_in[:], u_sb[:])
                                nc.gpsimd.collective_compute(
                                    "AllReduce",
                                    mybir.AluOpType.add,
                                    replica_groups=RG,
                                    ins=[u_in.opt()],
                                    outs=[u_out.opt()],
                                )
                                nc.sync.dma_start(uga_sb[:], u_out[:])
                            csplit = max(0, NCHUNK // 3)
                            if ci < csplit:
                                nc.vector.scalar_tensor_tensor(
                                    out=cs_scratch[:],
                                    in0=e_sb[:, ci * 1024:(ci + 1) * 1024],
                                    scalar=1.0,
                                    in1=w_rep[:],
                                    op0=mybir.AluOpType.mult,
                                    op1=mybir.AluOpType.mult,
                                    accum_out=colsum_sb[:, ci:ci + 1],
                                )
                            else:
                                nc.vector.tensor_tensor(
                                    out=cs_scratch2[:],
                                    in0=e_sb[:, ci * 1024:(ci + 1) * 1024],
                                    in1=w_rep[:],
                                    op=mybir.AluOpType.mult,
                                )
                                nc.scalar.activation(
                                    cs_scratch3[:],
                                    cs_scratch2[:],
                                    AF.Copy,
                                    accum_out=colsum_sb[:, ci:ci + 1],
                                )
                    for t in range(4):
                        nc.scalar.copy(u2_sb[:, t * B:(t + 1) * B], u_ps[t][:])

            if stage == 3:
                nc.vector.tensor_copy(mo_sb[:], u2_sb[:])
                nc.sync.dma_start(out_mo[:], mo_sb[:])
                nc.sync.dma_start(out_us[:], colsum_sb[:])
                nc.sync.dma_start(out_la[:], colsum_sb[:])

            if stage >= 4:
                # ---------- AllReduce #2b: second u partial ----------
                nc.sync.dma_start(u2_in[:], u2_sb[:])
                nc.gpsimd.collective_compute(
                    "AllReduce",
                    mybir.AluOpType.add,
                    replica_groups=RG,
                    ins=[u2_in.opt()],
                    outs=[u2_out.opt()],
                )
                nc.sync.dma_start(ugb_sb[:], u2_out[:])
                nc.vector.tensor_tensor(
                    out=ug_sb[:], in0=uga_sb[:], in1=ugb_sb[:],
                    op=mybir.AluOpType.add,
                )

                # ---------- finalize (replicated on every core) ----------
                for t in range(4):
                    for hf in range(2):
                        h = 2 * t + hf
                        nc.vector.tensor_tensor(
                            out=ctxf_sb[hf * 64: hf * 64 + 64, t * B:(t + 1) * B],
                            in0=ug_sb[hf * 64: hf * 64 + 64, t * B:(t + 1) * B],
                            in1=recip_rep[hf * 64: hf * 64 + 64, h * B:(h + 1) * B],
                            op=mybir.AluOpType.mult,
                        )
                for dc in range(4):
                    nc.scalar.activation(
                        ctx_sb[:, dc * B:(dc + 1) * B],
                        ctxf_sb[:, dc * B:(dc + 1) * B],
                        AF.Identity,
                        bias=bv_sb[:, dc:dc + 1],
                    )
                with tc.tile_pool(name="psf", bufs=1, space="PSUM") as psf:
                    ao_ps = psf.tile([128, H], F32, tag="aops")
                    for oc in range(4):
                        for dc in range(4):
                            nc.tensor.matmul(
                                ao_ps[:, oc * B:(oc + 1) * B],
                                lhsT=wo_sb[
                                    :, dc * H + oc * 128: dc * H + oc * 128 + 128
                                ],
                                rhs=ctx_sb[:, dc * B:(dc + 1) * B],
                                start=(dc == 0),
                                stop=(dc == 3),
                            )
                    for oc in range(4):
                        nc.scalar.activation(
                            ao_sb[:, oc * B:(oc + 1) * B],
                            ao_ps[:, oc * B:(oc + 1) * B],
                            AF.Identity,
                            bias=bo_sb[:, oc:oc + 1],
                        )
                    g1_ps = psf.tile([GH, B], F32, tag="g1ps")
                    for jc in range(4):
                        nc.tensor.matmul(
                            g1_ps[:],
                            lhsT=gw1_sb[:, jc * GH:(jc + 1) * GH],
                            rhs=qry_sb[:, jc * B:(jc + 1) * B],
                            start=(jc == 0),
                            stop=(jc == 3),
                        )
                    nc.scalar.activation(g1_sb[:], g1_ps[:], AF.Relu, bias=gb1_sb[:])
                    g2_ps = psf.tile([1, B], F32, tag="g2ps")
                    nc.tensor.matmul(g2_ps[:], lhsT=gw2_sb[:], rhs=g1_sb[:])
                    nc.scalar.activation(
                        gate_sb[:], g2_ps[:], AF.Sigmoid, bias=gb2_sb[:]
                    )
                nc.gpsimd.partition_broadcast(gate_rep[:], gate_sb[:])
                for oc in range(4):
                    nc.vector.tensor_tensor(
                        out=mo_sb[:, oc * B:(oc + 1) * B],
                        in0=ao_sb[:, oc * B:(oc + 1) * B],
                        in1=gate_rep[:],
                        op=mybir.AluOpType.mult,
                    )
                nc.sync.dma_start(out_mo[:], mo_sb[:])

                # usage / last updates for the local shard
                nc.vector.tensor_tensor(
                    out=ou_sb[:], in0=us_sb[:], in1=colsum_sb[:],
                    op=mybir.AluOpType.add,
                )
                nc.sync.dma_start(out_us[:], ou_sb[:])
                nc.vector.tensor_scalar(
                    out=mask_sb[:], in0=colsum_sb[:], scalar1=1e-3, scalar2=None,
                    op0=mybir.AluOpType.is_gt,
                )
                nc.vector.tensor_scalar(
                    out=srow_sb[:], in0=orow_sb[:], scalar1=step_sb[:, 0:1],
                    scalar2=None, op0=mybir.AluOpType.mult,
                )
                nc.gpsimd.partition_broadcast(srep_sb[:], srow_sb[:])
                nc.vector.select(ol_sb[:], mask_sb[:], srep_sb[:], la_sb[:])
                nc.sync.dma_start(out_la[:], ol_sb[:])

    nc.compile()
    return nc


def _get_nc():
    if "nc" not in _CACHE:
        _CACHE["nc"] = _build()
    return _CACHE["nc"]


def _prep_core(m, query, memory_keys, memory_values, in_proj_w, in_proj_b,
               out_proj_w, out_proj_b, gate_w1, gate_b1, gate_w2, gate_b2,
               usage_count, last_used, step_counter):
    s = slice(m * CS, (m + 1) * CS)

    def t_blocks(x):  # [CS, 512] -> [NBIG, 4, 128, 512] transposed blocks
        return np.ascontiguousarray(
            x[s].astype(BF16).reshape(NBIG, 512, 4, 128).transpose(0, 2, 3, 1)
        )

    def t_blocks8(x):  # [CS, 512] -> [NBIG, 2, 128, 2, 512] fp8 DR blocks, x32
        return np.ascontiguousarray(
            (x[s] * 32.0).astype(FP8)
            .reshape(NBIG, 512, 2, 2, 128).transpose(0, 2, 4, 3, 1)
        )

    return {
        "mk8": t_blocks8(memory_keys),
        "mvT": t_blocks(memory_values),
        "queryT": np.ascontiguousarray(query.T.astype(BF16).reshape(4, 128, B)),
        "wqT": np.ascontiguousarray(in_proj_w[:H].T.astype(BF16).reshape(4, 128, H)),
        "wkN": np.ascontiguousarray(
            in_proj_w[H:2 * H].astype(BF16).reshape(4, 128, H)
        ),
        "wvT": np.ascontiguousarray(
            in_proj_w[2 * H:].T.astype(BF16).reshape(4, 128, H)
        ),
        "woT": np.ascontiguousarray(out_proj_w.T.astype(BF16).reshape(4, 128, H)),
        "gw1T": np.ascontiguousarray(gate_w1.T.astype(BF16).reshape(4, 128, GH)),
        "gw2T": np.ascontiguousarray(gate_w2.T.astype(BF16).reshape(GH, 1)),
        "bq": np.ascontiguousarray(in_proj_b[:H].astype(np.float32).reshape(4, 128, 1)),
        "bv": np.ascontiguousarray(
            in_proj_b[2 * H:].astype(np.float32).reshape(4, 128, 1)
        ),
        "bo": np.ascontiguousarray(out_proj_b.astype(np.float32).reshape(4, 128, 1)),
        "gb1": np.ascontiguousarray(gate_b1.astype(np.float32).reshape(GH, 1)),
        "gb2": np.ascontiguousarray(gate_b2.astype(np.float32).reshape(1, 1)),
        "stepc": np.array([[np.float32(step_counter)]], dtype=np.float32),
        "usage": np.ascontiguousarray(
            usage_count[s].astype(np.float32).reshape(NCHUNK, 128).T
        ),
        "last": np.ascontiguousarray(
            last_used[s].astype(np.float32).reshape(NCHUNK, 128).T
        ),
    }


def kernel(**inputs):
    from concourse.bass_utils import run_bass_kernel_spmd

    nc = _get_nc()
    inputs = {k: np.asarray(v) for k, v in inputs.items()}
    in_maps = [_prep_core(m, **inputs) for m in range(NCORES)]
    res = run_bass_kernel_spmd(nc, in_maps, list(range(NCORES)))

    r0 = res.results[0]
    mo_t = r0["out_mo"]  # [128 o-local, 4*128 b]
    mo = np.empty((B, H), np.float32)
    for oc in range(4):
        mo[:, oc * 128:(oc + 1) * 128] = mo_t[:, oc * B:(oc + 1) * B].T
    new_usage = np.concatenate(
        [res.results[m]["out_us"].T.reshape(CS) for m in range(NCORES)]
    )
    new_last = np.concatenate(
        [res.results[m]["out_la"].T.reshape(CS) for m in range(NCORES)]
    )
    return mo, new_usage, new_last


# revision 25
# speedup vs baseline: 1.1089x; 1.1089x over previous
"""Distributed Trainium2 kernel for nn_AssociativeMemoryBank.

Math (reference):
  q = query @ Wq^T + bq                       [B, H]    -> heads [B, 8, 64]
  k = MK @ Wk^T + bk ; v = MV @ Wv^T + bv     [C, H]
  scores = q.k / 8 ; attn = softmax_c(scores) [B, 8, C]
  ctx = attn @ v ; attn_out = ctx @ Wo^T + bo [B, H]
  avg_attn = attn.mean(b, h)                  [C]
  new_usage = usage + avg_attn; new_last = where(avg_attn > 1e-3, step, last)
  out = sigmoid(relu(query@Gw1^T+g1)@Gw2^T+g2) * attn_out

Sharding: capacity axis (C=65536) split across 8 cores (8192 each).
Per core, one pass over the K-shard computes E^T = exp(scores^T) ([c,(h,b)],
SBUF-resident bf16; scores are ~N(0, 0.002) so no max-subtraction is needed),
plus the local sumexp via a ones-vector matmul.  AllReduce #1 (4KB) combines
sumexp.  A second pass over the V-shard accumulates the unnormalized
attn-weighted value sums u = E^T-weighted V (PSUM-resident) and the
1/sumexp-weighted per-slot attention column sums (usage stats).  AllReduce #2
(256KB) combines u; the tiny output projection + gate MLP run replicated.

Note bk cancels exactly (it shifts each (b,h) score row by a constant, which
softmax removes), so it is dropped.  bv enters as +bv (attn sums to 1).

Host-side prep is layout/dtype only (shard, transpose, bf16-cast); all FLOPs
over the big tensors run on device.
"""

import numpy as np
import ml_dtypes

B = 128
CAP = 65536
H = 512
NH = 8
HD = 64
GH = 128
NCORES = 8
CS = CAP // NCORES       # 8192 capacity slots per core
NCHUNK = CS // 128       # 64  c-chunks of 128
NBIG = CS // 512         # 16  c-chunks of 512
BF16 = ml_dtypes.bfloat16
FP8 = ml_dtypes.float8_e4m3

_CACHE = {}


def _build(stage=9):
    import concourse.bacc as bacc
    import concourse.mybir as mybir
    import concourse.tile as tile

    dt = mybir.dt
    F32 = dt.float32
    BF = dt.bfloat16
    F8 = dt.float8e4
    AF = mybir.ActivationFunctionType
    RG = [list(range(NCORES))]

    nc = bacc.Bacc(
        "TRN2", target_bir_lowering=False, debug=False, num_devices=NCORES
    )

    # ---- external inputs (per-core shards / replicated smalls) ----
    mk8 = nc.declare_dram_parameter("mk8", [NBIG, 2, 128, 2, 512], F8, isOutput=False)
    mvT = nc.declare_dram_parameter("mvT", [NBIG, 4, 128, 512], BF, isOutput=False)
    queryT = nc.declare_dram_parameter("queryT", [4, 128, B], BF, isOutput=False)
    wqT = nc.declare_dram_parameter("wqT", [4, 128, H], BF, isOutput=False)
    wkN = nc.declare_dram_parameter("wkN", [4, 128, H], BF, isOutput=False)
    wvT = nc.declare_dram_parameter("wvT", [4, 128, H], BF, isOutput=False)
    woT = nc.declare_dram_parameter("woT", [4, 128, H], BF, isOutput=False)
    gw1T = nc.declare_dram_parameter("gw1T", [4, 128, GH], BF, isOutput=False)
    gw2T = nc.declare_dram_parameter("gw2T", [GH, 1], BF, isOutput=False)
    bq = nc.declare_dram_parameter("bq", [4, 128, 1], F32, isOutput=False)
    bv = nc.declare_dram_parameter("bv", [4, 128, 1], F32, isOutput=False)
    bo = nc.declare_dram_parameter("bo", [4, 128, 1], F32, isOutput=False)
    gb1 = nc.declare_dram_parameter("gb1", [GH, 1], F32, isOutput=False)
    gb2 = nc.declare_dram_parameter("gb2", [1, 1], F32, isOutput=False)
    stepc = nc.declare_dram_parameter("stepc", [1, 1], F32, isOutput=False)
    usage = nc.declare_dram_parameter("usage", [128, NCHUNK], F32, isOutput=False)
    last = nc.declare_dram_parameter("last", [128, NCHUNK], F32, isOutput=False)

    out_mo = nc.declare_dram_parameter("out_mo", [128, H], F32, isOutput=True)
    out_us = nc.declare_dram_parameter("out_us", [128, NCHUNK], F32, isOutput=True)
    out_la = nc.declare_dram_parameter("out_la", [128, NCHUNK], F32, isOutput=True)

    with tile.TileContext(nc) as tc:
        with (
            tc.tile_pool(name="const", bufs=1) as const,
            tc.tile_pool(name="dram", bufs=1, space="DRAM") as dram,
        ):
            # ---------- persistent SBUF ----------
            e_sb = const.tile([128, NCHUNK * 1024], BF, tag="E")   # E^T, 128KB/p
            wkN_sb = const.tile([128, 4 * H], BF, tag="wkN")
            a8_sb = const.tile([128, 4 * NH * B], F8, tag="a8")
            wv_sb = const.tile([128, 4 * H], BF, tag="wv")
            wo_sb = const.tile([128, 4 * H], BF, tag="wo")
            gw1_sb = const.tile([128, 4 * GH], BF, tag="gw1")
            gw2_sb = const.tile([GH, 1], BF, tag="gw2")
            qry_sb = const.tile([128, 4 * B], BF, tag="qry")
            qT_sb = const.tile([128, 4 * B], BF, tag="qT")
            qbd2_sb = const.tile([128, 4 * 256], BF, tag="qbd2")
            ones_col = const.tile([128, 1], BF, tag="ones")
            bq_sb = const.tile([128, 4], F32, tag="bq")
            bq8_sb = const.tile([128, 4], F32, tag="bq8")
            bv_sb = const.tile([128, 4], F32, tag="bv")
            bo_sb = const.tile([128, 4], F32, tag="bo")
            gb1_sb = const.tile([GH, 1], F32, tag="gb1")
            gb2_sb = const.tile([1, 1], F32, tag="gb2")
            step_sb = const.tile([1, 1], F32, tag="step")
            us_sb = const.tile([128, NCHUNK], F32, tag="us")
            la_sb = const.tile([128, NCHUNK], F32, tag="la")
            colsum_sb = const.tile([128, NCHUNK], F32, tag="colsum")
            esum_sb = const.tile([1, NH * B], F32, tag="esum")
            esumg_sb = const.tile([1, NH * B], F32, tag="esumg")
            recip_sb = const.tile([1, NH * B], F32, tag="recip")
            recip_rep = const.tile([128, NH * B], F32, tag="reciprep")
            w_rep = const.tile([128, NH * B], BF, tag="wrep")
            u_sb = const.tile([128, H], BF, tag="u")
            u2_sb = const.tile([128, H], BF, tag="u2")
            uga_sb = const.tile([128, H], BF, tag="uga")
            ugb_sb = const.tile([128, H], BF, tag="ugb")
            ug_sb = const.tile([128, H], BF, tag="ug")
            ctxf_sb = const.tile([128, H], F32, tag="ctxf")
            ctx_sb = const.tile([128, H], BF, tag="ctx")
            ao_sb = const.tile([128, H], F32, tag="ao")
            mo_sb = const.tile([128, H], F32, tag="mo")
            g1_sb = const.tile([GH, B], BF, tag="g1")
            gate_sb = const.tile([1, B], F32, tag="gate")
            gate_rep = const.tile([128, B], F32, tag="gaterep")
            mask_sb = const.tile([128, NCHUNK], dt.uint8, tag="mask")
            orow_sb = const.tile([1, NCHUNK], F32, tag="orow")
            srow_sb = const.tile([1, NCHUNK], F32, tag="srow")
            srep_sb = const.tile([128, NCHUNK], F32, tag="srep")
            ol_sb = const.tile([128, NCHUNK], F32, tag="ol")
            ou_sb = const.tile([128, NCHUNK], F32, tag="ou")
            cs_scratch = const.tile([128, NH * B], BF, tag="csscratch")
            cs_scratch2 = const.tile([128, NH * B], BF, tag="csscratch2")
            cs_scratch3 = const.tile([128, NH * B], BF, tag="csscratch3")

            # DRAM bounce buffers for the collectives
            esum_in = dram.tile([1, NH * B], F32)
            esum_out = dram.tile([1, NH * B], F32, addr_space="Shared")
            u_in = dram.tile([128, H], BF)
            u_out = dram.tile([128, H], BF, addr_space="Shared")
            u2_in = dram.tile([128, H], BF)
            u2_out = dram.tile([128, H], BF, addr_space="Shared")

            # ---------- load statics (SWDGE queue; HWDGE leads with K chunks) --
            for dc in range(4):
                nc.sync.dma_start(wkN_sb[:, dc * H:(dc + 1) * H], wkN[dc])
            for jc in range(4):
                nc.sync.dma_start(qry_sb[:, jc * B:(jc + 1) * B], queryT[jc])
                nc.sync.dma_start(bq_sb[:, jc:jc + 1], bq[jc])
                nc.gpsimd.dma_start(wv_sb[:, jc * H:(jc + 1) * H], wvT[jc])
                nc.gpsimd.dma_start(wo_sb[:, jc * H:(jc + 1) * H], woT[jc])
                nc.gpsimd.dma_start(gw1_sb[:, jc * GH:(jc + 1) * GH], gw1T[jc])
                nc.gpsimd.dma_start(bv_sb[:, jc:jc + 1], bv[jc])
                nc.gpsimd.dma_start(bo_sb[:, jc:jc + 1], bo[jc])
            nc.gpsimd.dma_start(gw2_sb[:], gw2T[:])
            nc.gpsimd.dma_start(gb1_sb[:], gb1[:])
            nc.gpsimd.dma_start(gb2_sb[:], gb2[:])
            nc.gpsimd.dma_start(step_sb[:], stepc[:])
            nc.gpsimd.dma_start(us_sb[:], usage[:])
            nc.gpsimd.dma_start(la_sb[:], last[:])
            nc.vector.memset(ones_col[:], 1.0)
            nc.vector.memset(orow_sb[:], 1.0)
            nc.vector.tensor_scalar_mul(bq8_sb[:], bq_sb[:], 0.125)

            # ---------- setup: q^T = (Wq @ query^T + bq)/8, bf16 ----------
            with (
                tc.tile_pool(name="psq", bufs=2, space="PSUM") as psq,
                tc.tile_pool(name="setup_sb", bufs=1) as setup_sb,
            ):
                wq_sb = setup_sb.tile([128, 4 * H], BF, tag="wq")
                for jc in range(4):
                    nc.sync.dma_start(
                        wq_sb[:, jc * H:(jc + 1) * H], wqT[jc]
                    )
                for dc in range(4):
                    q_ps = psq.tile([128, B], F32, tag="qps")
                    for jc in range(4):
                        nc.tensor.matmul(
                            q_ps[:],
                            lhsT=wq_sb[:, jc * H + dc * 128: jc * H + dc * 128 + 128],
                            rhs=qry_sb[:, jc * B:(jc + 1) * B],
                            start=(jc == 0),
                            stop=(jc == 3),
                        )
                    nc.scalar.activation(
                        qT_sb[:, dc * B:(dc + 1) * B],
                        q_ps[:],
                        AF.Identity,
                        bias=bq8_sb[:, dc:dc + 1],
                        scale=0.125,
                    )
            # block-diag packed q for head pairs: pair t cols [t*256, t*256+256)
            nc.vector.memset(qbd2_sb[:], 0.0)
            for p in range(4):
                nc.vector.tensor_copy(
                    qbd2_sb[0:64, p * 256: p * 256 + B],
                    qT_sb[0:64, p * B:(p + 1) * B],
                )
                nc.vector.tensor_copy(
                    qbd2_sb[64:128, p * 256 + B: p * 256 + 2 * B],
                    qT_sb[64:128, p * B:(p + 1) * B],
                )
            # A^T[j, (h,b)] = (Wk^T q)/8 : rows j, cols (h, b); fp8 x32
            with tc.tile_pool(name="psa", bufs=2, space="PSUM") as psa:
                for jc in range(4):
                    a_ps = psa.tile([128, NH * B], F32, tag="aps")
                    for t in range(4):
                        nc.tensor.matmul(
                            a_ps[:, t * 256:(t + 1) * 256],
                            lhsT=wkN_sb[:, t * H + jc * 128: t * H + jc * 128 + 128],
                            rhs=qbd2_sb[:, t * 256:(t + 1) * 256],
                            start=True,
                            stop=True,
                        )
                    nc.vector.tensor_scalar_mul(
                        a8_sb[:, jc * NH * B:(jc + 1) * NH * B], a_ps[:], 32.0
                    )

            # ---------- phase B1: K-projection, scores^T, E^T, local sumexp ----
            with (
                tc.tile_pool(name="pmk", bufs=3) as pmk,
                tc.tile_pool(name="pss", bufs=2, space="PSUM") as pss,
                tc.tile_pool(name="pse", bufs=1, space="PSUM") as pse,
            ):
                esum_ps = pse.tile([1, NH * B], F32, tag="esumps")
                for cc in range(NBIG):
                    mk_sb = pmk.tile([128, 2 * 1024], F8, tag="mk")
                    for jp in range(2):
                        nc.sync.dma_start(
                            mk_sb[:, jp * 1024:(jp + 1) * 1024], mk8[cc, jp]
                        )
                    for ci4 in range(4):
                        ci = cc * 4 + ci4
                        for hf in range(2):
                            s_ps = pss.tile([128, 512], F32, tag="sps")
                            for jp in range(2):
                                nc.tensor.matmul(
                                    s_ps[:],
                                    lhsT=mk_sb[
                                        :, jp * 1024:(jp + 1) * 1024
                                    ].rearrange("p (s c) -> p s c", s=2)[
                                        :, :, ci4 * 128:(ci4 + 1) * 128
                                    ],
                                    rhs=a8_sb.rearrange(
                                        "p (j s n) -> p j s n", j=2, s=2
                                    )[:, jp, :, hf * 512:(hf + 1) * 512],
                                    start=(jp == 0),
                                    stop=(jp == 1),
                                    perf_mode=mybir.MatmulPerfMode.DoubleRow,
                                )
                            nc.scalar.activation(
                                e_sb[
                                    :,
                                    ci * 1024 + hf * 512: ci * 1024 + hf * 512 + 512,
                                ],
                                s_ps[:],
                                AF.Exp,
                                scale=1.0 / 1024.0,
                            )
                        for hf in range(2):
                            nc.tensor.matmul(
                                esum_ps[:, hf * 512: hf * 512 + 512],
                                lhsT=ones_col[:],
                                rhs=e_sb[
                                    :,
                                    ci * 1024 + hf * 512: ci * 1024 + hf * 512 + 512,
                                ],
                                start=(ci == 0),
                                stop=(ci == NCHUNK - 1),
                                skip_group_check=True,
                            )
                nc.vector.tensor_copy(esum_sb[:], esum_ps[:])

            if stage < 2:
                nc.vector.memset(mo_sb[:], 0.0)
                nc.vector.tensor_copy(mo_sb[0:1, 0:H], esum_sb[:, 0:H])
                nc.sync.dma_start(out_mo[:], mo_sb[:])
                nc.vector.memset(ou_sb[:], 0.0)
                nc.sync.dma_start(out_us[:], ou_sb[:])
                nc.sync.dma_start(out_la[:], ou_sb[:])

            if stage >= 2:
                # ---------- AllReduce #1: sumexp ----------
                nc.sync.dma_start(esum_in[:], esum_sb[:])
                nc.gpsimd.collective_compute(
                    "AllReduce",
                    mybir.AluOpType.add,
                    replica_groups=RG,
                    ins=[esum_in.opt()],
                    outs=[esum_out.opt()],
                )
                nc.sync.dma_start(esumg_sb[:], esum_out[:])
                nc.vector.reciprocal(recip_sb[:], esumg_sb[:])
                nc.gpsimd.partition_broadcast(recip_rep[:], recip_sb[:])
                nc.vector.tensor_scalar_mul(w_rep[:], recip_rep[:], 1.0 / (B * NH))

            if stage == 2:
                nc.vector.memset(mo_sb[:], 0.0)
                nc.vector.tensor_copy(mo_sb[0:1, 0:H], esumg_sb[:, 0:H])
                nc.sync.dma_start(out_mo[:], mo_sb[:])
                nc.vector.memset(ou_sb[:], 0.0)
                nc.sync.dma_start(out_us[:], ou_sb[:])
                nc.sync.dma_start(out_la[:], ou_sb[:])

            if stage >= 3:
                # ---------- phase B2: V-projection, u accum, colsum ----------
                with (
                    tc.tile_pool(name="pmv", bufs=3) as pmv,
                    tc.tile_pool(name="pvt", bufs=3) as pvt,
                    tc.tile_pool(name="psv", bufs=2, space="PSUM") as psv,
                    tc.tile_pool(name="psu", bufs=1, space="PSUM") as psu,
                ):
                    u_ps = [
                        psu.tile([128, B], F32, tag=f"ups{t}", name=f"ups{t}")
                        for t in range(4)
                    ]
                    for cc in range(NBIG):
                        mv_sb = pmv.tile([128, 4 * 512], BF, tag="mv")
                        for jc in range(4):
                            nc.sync.dma_start(
                                mv_sb[:, jc * 512:(jc + 1) * 512], mvT[cc, jc]
                            )
                        for ci4 in range(4):
                            ci = cc * 4 + ci4
                            v_ps = psv.tile([128, 512], F32, tag="vps")
                            for jc in range(4):
                                nc.tensor.matmul(
                                    v_ps[:],
                                    lhsT=mv_sb[
                                        :,
                                        jc * 512 + ci4 * 128: jc * 512 + ci4 * 128
                                        + 128,
                                    ],
                                    rhs=wv_sb[:, jc * H:(jc + 1) * H],
                                    start=(jc == 0),
                                    stop=(jc == 3),
                                )
                            v_sb = pvt.tile([128, 512], BF, tag="vt")
                            nc.scalar.copy(v_sb[:], v_ps[:])
                            usplit = max(1, (NCHUNK * 3) // 4)
                            for h in range(NH):
                                nc.tensor.matmul(
                                    u_ps[h // 2][(h % 2) * 64:(h % 2) * 64 + 64, :],
                                    lhsT=v_sb[:, h * 64: h * 64 + 64],
                                    rhs=e_sb[
                                        :, ci * 1024 + h * B: ci * 1024 + (h + 1) * B
                                    ],
                                    start=(ci == 0 or ci == usplit),
                                    stop=(ci == usplit - 1 or ci == NCHUNK - 1),
                                    skip_group_check=True,
                                )
                            if ci == usplit - 1:
                                # first partial AllReduce, overlapped with the rest
                                for t in range(4):
                                    nc.scalar.copy(
                                        u_sb[:, t * B:(t + 1) * B], u_ps[t][:]
                                    )
                                nc.sync.dma_start(u_in[:], u_sb[:])
                                nc.gpsimd.collective_compute(
                                    "AllReduce",
                                    mybir.AluOpType.add,
                                    replica_groups=RG,
                                    ins=[u_in.opt()],
                                    outs=[u_out.opt()],
                                )
                                nc.sync.dma_start(uga_sb[:], u_out[:])
                            csplit = max(0, NCHUNK // 3)
                            if ci < csplit:
                                nc.vector.scalar_tensor_tensor(
                                    out=cs_scratch[:],
                                    in0=e_sb[:, ci * 1024:(ci + 1) * 1024],
                                    scalar=1.0,
                                    in1=w_rep[:],
                                    op0=mybir.AluOpType.mult,
                                    op1=mybir.AluOpType.mult,
                                    accum_out=colsum_sb[:, ci:ci + 1],
                                )
                            else:
                                nc.vector.tensor_tensor(
                                    out=cs_scratch2[:],
                                    in0=e_sb[:, ci * 1024:(ci + 1) * 1024],
                                    in1=w_rep[:],
                                    op=mybir.AluOpType.mult,
                                )
                                nc.scalar.activation(
                                    cs_scratch3[:],
                                    cs_scratch2[:],
                                    AF.Copy,
                                    accum_out=colsum_sb[:, ci:ci + 1],
                                )
                    for t in range(4):
                        nc.scalar.copy(u2_sb[:, t * B:(t + 1) * B], u_ps[t][:])

            if stage == 3:
                nc.vector.tensor_copy(mo_sb[:], u2_sb[:])
                nc.sync.dma_start(out_mo[:], mo_sb[:])
                nc.sync.dma_start(out_us[:], colsum_sb[:])
                nc.sync.dma_start(out_la[:], colsum_sb[:])

            if stage >= 4:
                # ---------- AllReduce #2b: second u partial ----------
                nc.sync.dma_start(u2_in[:], u2_sb[:])
                nc.gpsimd.collective_compute(
                    "AllReduce",
                    mybir.AluOpType.add,
                    replica_groups=RG,
                    ins=[u2_in.opt()],
                    outs=[u2_out.opt()],
                )
                nc.sync.dma_start(ugb_sb[:], u2_out[:])
                nc.vector.tensor_tensor(
                    out=ug_sb[:], in0=uga_sb[:], in1=ugb_sb[:],
                    op=mybir.AluOpType.add,
                )

                # ---------- finalize (replicated on every core) ----------
                for t in range(4):
                    for hf in range(2):
                        h = 2 * t + hf
                        nc.vector.tensor_tensor(
                            out=ctxf_sb[hf * 64: hf * 64 + 64, t * B:(t + 1) * B],
                            in0=ug_sb[hf * 64: hf * 64 + 64, t * B:(t + 1) * B],
                            in1=recip_rep[hf * 64: hf * 64 + 64, h * B:(h + 1) * B],
                            op=mybir.AluOpType.mult,
                        )
                for dc in range(4):
                    nc.scalar.activation(
                        ctx_sb[:, dc * B:(dc + 1) * B],
                        ctxf_sb[:, dc * B:(dc + 1) * B],
                        AF.Identity,
                        bias=bv_sb[:, dc:dc + 1],
                    )
                with tc.tile_pool(name="psf", bufs=1, space="PSUM") as psf:
                    ao_ps = psf.tile([128, H], F32, tag="aops")
                    for oc in range(4):
                        for dc in range(4):
                            nc.tensor.matmul(
                                ao_ps[:, oc * B:(oc + 1) * B],
                                lhsT=wo_sb[
                                    :, dc * H + oc * 128: dc * H + oc * 128 + 128
                                ],
                                rhs=ctx_sb[:, dc * B:(dc + 1) * B],
                                start=(dc == 0),
                                stop=(dc == 3),
                            )
                    for oc in range(4):
                        nc.scalar.activation(
                            ao_sb[:, oc * B:(oc + 1) * B],
                            ao_ps[:, oc * B:(oc + 1) * B],
                            AF.Identity,
                            bias=bo_sb[:, oc:oc + 1],
                        )
                    g1_ps = psf.tile([GH, B], F32, tag="g1ps")
                    for jc in range(4):
                        nc.tensor.matmul(
                            g1_ps[:],
                            lhsT=gw1_sb[:, jc * GH:(jc + 1) * GH],
                            rhs=qry_sb[:, jc * B:(jc + 1) * B],
                            start=(jc == 0),
                            stop=(jc == 3),
                        )
                    nc.scalar.activation(g1_sb[:], g1_ps[:], AF.Relu, bias=gb1_sb[:])
                    g2_ps = psf.tile([1, B], F32, tag="g2ps")
                    nc.tensor.matmul(g2_ps[:], lhsT=gw2_sb[:], rhs=g1_sb[:])
                    nc.scalar.activation(
                        gate_sb[:], g2_ps[:], AF.Sigmoid, bias=gb2_sb[:]
                    )
                nc.gpsimd.partition_broadcast(gate_rep[:], gate_sb[:])
                for oc in range(4):
                    nc.vector.tensor_tensor(
                        out=mo_sb[:, oc * B:(oc + 1) * B],
                        in0=ao_sb[:, oc * B:(oc + 1) * B],
                        in1=gate_rep[:],
                        op=mybir.AluOpType.mult,
                    )
                nc.sync.dma_start(out_mo[:], mo_sb[:])

                # usage / last updates for the local shard
                nc.vector.tensor_tensor(
                    out=ou_sb[:], in0=us_sb[:], in1=colsum_sb[:],
                    op=mybir.AluOpType.add,
                )
                nc.sync.dma_start(out_us[:], ou_sb[:])
                nc.vector.tensor_scalar(
                    out=mask_sb[:], in0=colsum_sb[:], scalar1=1e-3, scalar2=None,
                    op0=mybir.AluOpType.is_gt,
                )
                nc.vector.tensor_scalar(
                    out=srow_sb[:], in0=orow_sb[:], scalar1=step_sb[:, 0:1],
                    scalar2=None, op0=mybir.AluOpType.mult,
                )
                nc.gpsimd.partition_broadcast(srep_sb[:], srow_sb[:])
                nc.vector.select(ol_sb[:], mask_sb[:], srep_sb[:], la_sb[:])
                nc.sync.dma_start(out_la[:], ol_sb[:])

    nc.compile()
    return nc


def _get_nc():
    if "nc" not in _CACHE:
        _CACHE["nc"] = _build()
    return _CACHE["nc"]


def _prep_core(m, query, memory_keys, memory_values, in_proj_w, in_proj_b,
               out_proj_w, out_proj_b, gate_w1, gate_b1, gate_w2, gate_b2,
               usage_count, last_used, step_counter):
    s = slice(m * CS, (m + 1) * CS)

    def t_blocks(x):  # [CS, 512] -> [NBIG, 4, 128, 512] transposed blocks
        return np.ascontiguousarray(
            x[s].astype(BF16).reshape(NBIG, 512, 4, 128).transpose(0, 2, 3, 1)
        )

    def t_blocks8(x):  # [CS, 512] -> [NBIG, 2, 128, 2, 512] fp8 DR blocks, x32
        return np.ascontiguousarray(
            (x[s] * 32.0).astype(FP8)
            .reshape(NBIG, 512, 2, 2, 128).transpose(0, 2, 4, 3, 1)
        )

    return {
        "mk8": t_blocks8(memory_keys),
        "mvT": t_blocks(memory_values),
        "queryT": np.ascontiguousarray(query.T.astype(BF16).reshape(4, 128, B)),
        "wqT": np.ascontiguousarray(in_proj_w[:H].T.astype(BF16).reshape(4, 128, H)),
        "wkN": np.ascontiguousarray(
            in_proj_w[H:2 * H].astype(BF16).reshape(4, 128, H)
        ),
        "wvT": np.ascontiguousarray(
            in_proj_w[2 * H:].T.astype(BF16).reshape(4, 128, H)
        ),
        "woT": np.ascontiguousarray(out_proj_w.T.astype(BF16).reshape(4, 128, H)),
        "gw1T": np.ascontiguousarray(gate_w1.T.astype(BF16).reshape(4, 128, GH)),
        "gw2T": np.ascontiguousarray(gate_w2.T.astype(BF16).reshape(GH, 1)),
        "bq": np.ascontiguousarray(in_proj_b[:H].astype(np.float32).reshape(4, 128, 1)),
        "bv": np.ascontiguousarray(
            in_proj_b[2 * H:].astype(np.float32).reshape(4, 128, 1)
        ),
        "bo": np.ascontiguousarray(out_proj_b.astype(np.float32).reshape(4, 128, 1)),
        "gb1": np.ascontiguousarray(gate_b1.astype(np.float32).reshape(GH, 1)),
        "gb2": np.ascontiguousarray(gate_b2.astype(np.float32).reshape(1, 1)),
        "stepc": np.array([[np.float32(step_counter)]], dtype=np.float32),
        "usage": np.ascontiguousarray(
            usage_count[s].astype(np.float32).reshape(NCHUNK, 128).T
        ),
        "last": np.ascontiguousarray(
            last_used[s].astype(np.float32).reshape(NCHUNK, 128).T
        ),
    }


def kernel(**inputs):
    from concourse.bass_utils import run_bass_kernel_spmd

    nc = _get_nc()
    inputs = {k: np.asarray(v) for k, v in inputs.items()}
    in_maps = [_prep_core(m, **inputs) for m in range(NCORES)]
    res = run_bass_kernel_spmd(nc, in_maps, list(range(NCORES)))

    r0 = res.results[0]
    mo_t = r0["out_mo"]  # [128 o-local, 4*128 b]
    mo = np.empty((B, H), np.float32)
    for oc in range(4):
        mo[:, oc * 128:(oc + 1) * 128] = mo_t[:, oc * B:(oc + 1) * B].T
    new_usage = np.concatenate(
        [res.results[m]["out_us"].T.reshape(CS) for m in range(NCORES)]
    )
    new_last = np.concatenate(
        [res.results[m]["out_la"].T.reshape(CS) for m in range(NCORES)]
    )
    return mo, new_usage, new_last
